# revision 18
# baseline (speedup 1.0000x reference)
"""Trainium2 Bass kernel for nn_CustomModel_13657996001613 (moe_routing).

Distribution: data-parallel over nodes (1024 nodes -> 8 cores x 128).
GCN segment-sums use host-bucketed edges folded (with the degree
normalization) into 128x128 adjacency blocks, plus one AllGather of the
per-shard GCN features per GCN layer.

Host precomputes the weight-only reductions (sum_l wm1 / class2 / wm2)
and packs all device inputs into a handful of [128, W] buffers so the
kernel issues only a few large DMAs.  The logits MLP runs in fp16 (its
argmax decides routing; fp16 noise is well under the observed top-2
logit gaps); everything else off the argmax path is bf16.  The final
log_softmax normalization happens on host (rowwise, exact).
"""
import sys

if "/opt/trn_rl_repo" not in sys.path:
    sys.path.insert(0, "/opt/trn_rl_repo")

import ml_dtypes
import numpy as np

import concourse.bass as bass  # noqa: F401
import concourse.mybir as mybir
import concourse.tile as tile
from concourse import bacc, bass_utils
from concourse.masks import make_identity

F32 = mybir.dt.float32
F16 = mybir.dt.float16
BF16 = mybir.dt.bfloat16
NPBF16 = ml_dtypes.bfloat16
OP = mybir.AluOpType
AF = mybir.ActivationFunctionType
AX = mybir.AxisListType

NCORE = 8
P = 128
NNODE = 1024
D = 768
KX = D // P      # 6
H = 512
HT = H // P      # 4
F1 = 256
F2 = 32
TAU = 0.7
RLO = (1.0 - TAU) / 2.0       # 0.15
RDELT = TAU - RLO             # 0.55

# ---- packed-buffer column offsets (shared between host and device) ----
# pma (f16): x1fm | mW1 | x2nm     pmb (f16): mW2 | mW3
PMA_X1, PMA_W1 = 0, 768
PMA_X2 = PMA_W1 + KX * H
PMA_END = PMA_X2 + 768
PMB_W2, PMB_W3, PMB_END = 0, HT * H, HT * H + HT * 3
# pga (bf16): x11fm | gW1          pgb (bf16): gW2 | gW3
PGA_X11, PGA_W1, PGA_END = 0, 768, 768 + KX * H
PGB_W2, PGB_W3, PGB_END = 0, HT * H, HT * H + HT * 2
# pr (bf16): g1W | w13 | ablk | g2W
PR_G1W = 0
PR_W13 = PR_G1W + KX * F1
PR_ABLK = PR_W13 + KX * F1
PR_G2W = PR_ABLK + NCORE * P
PR_END = PR_G2W + 2 * F2
# pc3 (bf16, [3, *]): bp2 | wm12 | W1s | W2s
C3_BP2, C3_WM12, C3_W1S, C3_W2S, C3_END = 0, 768, 1024, 1792, 2048
# pc2 (bf16, [2, *]): bp1 | C2s
C2_BP1, C2_C2S, C2_END = 0, 768, 1536
# crow (bf16, [1, *]): g1b | g2b | fcb | gb3 | mb1 | mb2 | gb1 | gb2 | mb3
CR_G1B, CR_G2B, CR_FCB, CR_GB3 = 0, 256, 288, 296
CR_MB1, CR_MB2, CR_GB1, CR_GB2, CR_MB3 = 304, 816, 1328, 1840, 2352
CR_END = 2355

N_WARM = 40      # PE warm-up matmuls at t=0
N_WARM1 = 24     # keep-warm matmuls during AllGather #1
N_WARM2 = 12     # keep-warm matmuls during AllGather #2

_module_cache: dict[int, "bacc.Bacc"] = {}


def _build(T: int) -> "bacc.Bacc":
    nc = bacc.Bacc(
        "TRN2",
        target_bir_lowering=False,
        debug=False,
        enable_asserts=False,
        num_devices=NCORE,
    )
    dt = F32

    pma_d = nc.dram_tensor("pma", [P, PMA_END], F16, kind="ExternalInput")
    pmb_d = nc.dram_tensor("pmb", [P, PMB_END], F16, kind="ExternalInput")
    pga_d = nc.dram_tensor("pga", [P, PGA_END], BF16, kind="ExternalInput")
    pgb_d = nc.dram_tensor("pgb", [P, PGB_END], BF16, kind="ExternalInput")
    pr_d = nc.dram_tensor("pr", [P, PR_END], BF16, kind="ExternalInput")
    pc3_d = nc.dram_tensor("pc3", [3, C3_END], BF16, kind="ExternalInput")
    pc2_d = nc.dram_tensor("pc2", [2, C2_END], BF16, kind="ExternalInput")
    crow_d = nc.dram_tensor("crow", [1, CR_END], BF16, kind="ExternalInput")
    fcw_d = nc.dram_tensor("fcw", [F2, 8], BF16, kind="ExternalInput")
    out_d = nc.dram_tensor("out", [P, 8], F32, kind="ExternalOutput")

    with tile.TileContext(nc) as tc:
        from contextlib import ExitStack

        ctx = ExitStack()
        with ctx:
            const = ctx.enter_context(tc.tile_pool(name="const", bufs=1))
            packs = ctx.enter_context(tc.tile_pool(name="packs", bufs=1))
            actp = ctx.enter_context(tc.tile_pool(name="actp", bufs=9))
            work = ctx.enter_context(tc.tile_pool(name="work", bufs=1))
            ps = ctx.enter_context(tc.tile_pool(name="ps", bufs=4, space="PSUM"))
            psw = ctx.enter_context(tc.tile_pool(name="psw", bufs=1, space="PSUM"))
            dpool = ctx.enter_context(
                tc.tile_pool(name="dram", bufs=1, space="DRAM")
            )

            # ---- constants ------------------------------------------------
            identh = const.tile([P, P], F16, tag="identh", name="identh")
            make_identity(nc, identh[:])
            identb = const.tile([P, P], BF16, tag="identb", name="identb")
            make_identity(nc, identb[:])
            onesb = const.tile([1, P], BF16, tag="onesb", name="onesb")
            nc.vector.memset(onesb[:], 1.0)
            junk = const.tile([1, 1], dt, tag="junk", name="junk")
            nc.vector.memset(junk[:], 0.25)

            # ---- small-constant DMAs (scalar queue) -----------------------
            pc3_sb = const.tile([3, C3_END], BF16, tag="pc3", name="pc3")
            nc.scalar.dma_start(pc3_sb[:], pc3_d[:, :])
            pc2_sb = const.tile([2, C2_END], BF16, tag="pc2", name="pc2")
            nc.scalar.dma_start(pc2_sb[:], pc2_d[:, :])
            crow_sb = const.tile([1, CR_END], BF16, tag="crow", name="crow")
            nc.scalar.dma_start(crow_sb[:], crow_d[:, :])
            fcw_sb = const.tile([F2, 8], BF16, tag="fcw", name="fcw")
            nc.scalar.dma_start(fcw_sb[:], fcw_d[:, :])

            # pre-load the Relu activation table while the queue is idle
            jout = work.tile([1, 1], dt, tag="jout", name="jout")
            nc.scalar.activation(jout[:], junk[:], AF.Relu)

            # ---- big input DMAs, dependency order (sync queue) ------------
            pga_sb = packs.tile([P, PGA_END], BF16, tag="pga", name="pga")
            nc.sync.dma_start(pga_sb[:], pga_d[:, :])
            pma_sb = packs.tile([P, PMA_END], F16, tag="pma", name="pma")
            nc.sync.dma_start(pma_sb[:], pma_d[:, :])
            pgb_sb = packs.tile([P, PGB_END], BF16, tag="pgb", name="pgb")
            nc.sync.dma_start(pgb_sb[:], pgb_d[:, :])
            pmb_sb = packs.tile([P, PMB_END], F16, tag="pmb", name="pmb")
            nc.sync.dma_start(pmb_sb[:], pmb_d[:, :])
            pr_sb = packs.tile([P, PR_END], BF16, tag="pr", name="pr")
            nc.sync.dma_start(pr_sb[:], pr_d[:, :])

            # ---- PE warm-up spam (runs while DMAs stream) -----------------
            spam = psw.tile([P, P], dt, tag="spam", name="spam")
            for _ in range(N_WARM):
                nc.tensor.matmul(spam[:], identb[:], identb[:])

            # ---- 2-hidden-layer MLP, node-major with transposes -----------
            # biases land in PSUM via a K=1 ones-row matmul, so each layer
            # needs only one big Relu ACT after the transposes
            def mlp_layer(xt, xoff, kx, woff, boff, pw, wdt, idn):
                ps1 = ps.tile([P, H], dt, tag="ps", name="ps")
                for k in range(kx):
                    nc.tensor.matmul(
                        ps1[:],
                        xt[:, xoff + k * P:xoff + (k + 1) * P],
                        pw[:, woff + k * H:woff + (k + 1) * H],
                        start=(k == 0), stop=False,
                    )
                nc.tensor.matmul(
                    ps1[:], onesb[:], crow_sb[:, boff:boff + H],
                    start=False, stop=True,
                )
                hn = actp.tile([P, H], wdt, tag="hn", name="hn")
                nc.vector.tensor_copy(hn[:], ps1[:])
                pst = ps.tile([P, H], wdt, tag="ps", name="ps")
                for m in range(HT):
                    nc.tensor.transpose(
                        pst[:, m * P:(m + 1) * P], hn[:, m * P:(m + 1) * P],
                        idn[:],
                    )
                ht = actp.tile([P, H], wdt, tag="ht", name="ht")
                nc.scalar.activation(ht[:], pst[:], AF.Relu)
                return ht

            def mlp_2layers(xoff, w1off, w2off, pa, pb, b1off, b2off, wdt, idn):
                h1 = mlp_layer(pa, xoff, KX, w1off, b1off, pa, wdt, idn)
                return mlp_layer(h1, 0, HT, w2off, b2off, pb, wdt, idn)

            # ---- branch g: x11 -> gT [2,128] bf16 -------------------------
            g_h2 = mlp_2layers(
                PGA_X11, PGA_W1, PGB_W2, pga_sb, pgb_sb,
                CR_GB1, CR_GB2, BF16, identb,
            )
            ps_g = ps.tile([2, P], dt, tag="ps", name="ps")
            for k in range(HT):
                nc.tensor.matmul(
                    ps_g[:],
                    pgb_sb[:, PGB_W3 + k * 2:PGB_W3 + (k + 1) * 2],
                    g_h2[:, k * P:(k + 1) * P],
                    start=(k == 0), stop=False,
                )
            nc.tensor.matmul(
                ps_g[:], crow_sb[:, CR_GB3:CR_GB3 + 2], onesb[:],
                start=False, stop=True,
            )
            gT_sb = work.tile([2, P], BF16, tag="gT", name="gT")
            nc.vector.tensor_copy(gT_sb[:], ps_g[:])

            # ---- branch logits (fp16): x1 -> r -> rT [3,128] bf16 ---------
            m_h2 = mlp_2layers(
                PMA_X1, PMA_W1, PMB_W2, pma_sb, pmb_sb,
                CR_MB1, CR_MB2, F16, identh,
            )
            ps_l = ps.tile([P, 3], dt, tag="ps", name="ps")
            for k in range(HT):
                nc.tensor.matmul(
                    ps_l[:], m_h2[:, k * P:(k + 1) * P],
                    pmb_sb[:, PMB_W3 + k * 3:PMB_W3 + (k + 1) * 3],
                    start=(k == 0), stop=False,
                )
            nc.tensor.matmul(
                ps_l[:], onesb[:], crow_sb[:, CR_MB3:CR_MB3 + 3],
                start=False, stop=True,
            )
            mx_sb = work.tile([P, 1], dt, tag="mx", name="mx")
            nc.vector.tensor_reduce(mx_sb[:], ps_l[:], axis=AX.X, op=OP.max)
            r_sb = work.tile([P, 3], BF16, tag="r", name="r")
            nc.vector.tensor_scalar(
                r_sb[:], ps_l[:], mx_sb[:, 0:1], None, OP.is_ge
            )
            nc.vector.tensor_scalar(
                r_sb[:], r_sb[:], RDELT, RLO, OP.mult, OP.add
            )
            ps_rt = ps.tile([3, P], BF16, tag="ps", name="ps")
            nc.tensor.transpose(ps_rt[:], r_sb[:], identb[:])
            rT_sb = work.tile([3, P], BF16, tag="rT", name="rT")
            nc.vector.tensor_copy(rT_sb[:], ps_rt[:])

            # ---- res1 chain, node-major, half-pipelined -------------------
            # res1 = ((C2s^T g) * x2 + bp1^T g) * (W1s^T r) + bp2^T r
            r1 = work.tile([P, D], dt, tag="r1", name="r1")
            r1b = work.tile([P, D], BF16, tag="r1b", name="r1b")
            r1t = []
            HW = 384
            for h_ in range(2):
                sl = slice(h_ * HW, (h_ + 1) * HW)
                psT1 = ps.tile([P, HW], dt, tag="ps", name="ps")
                nc.tensor.matmul(
                    psT1[:], gT_sb[:], pc2_sb[:, C2_C2S + h_ * HW:C2_C2S + (h_ + 1) * HW]
                )
                psT2 = ps.tile([P, HW], dt, tag="ps", name="ps")
                nc.tensor.matmul(
                    psT2[:], gT_sb[:], pc2_sb[:, C2_BP1 + h_ * HW:C2_BP1 + (h_ + 1) * HW]
                )
                psT3 = ps.tile([P, HW], dt, tag="ps", name="ps")
                nc.tensor.matmul(
                    psT3[:], rT_sb[:], pc3_sb[:, C3_W1S + h_ * HW:C3_W1S + (h_ + 1) * HW]
                )
                psT4 = ps.tile([P, HW], dt, tag="ps", name="ps")
                nc.tensor.matmul(
                    psT4[:], rT_sb[:], pc3_sb[:, C3_BP2 + h_ * HW:C3_BP2 + (h_ + 1) * HW]
                )
                nc.vector.tensor_tensor(
                    r1[:, sl], psT1[:], pma_sb[:, PMA_X2 + h_ * HW:PMA_X2 + (h_ + 1) * HW],
                    op=OP.mult,
                )
                nc.vector.tensor_tensor(r1[:, sl], r1[:, sl], psT2[:], op=OP.add)
                nc.vector.tensor_tensor(r1[:, sl], r1[:, sl], psT3[:], op=OP.mult)
                nc.vector.tensor_tensor(r1b[:, sl], r1[:, sl], psT4[:], op=OP.add)
                for k in range(h_ * 3, h_ * 3 + 3):
                    pst = ps.tile([P, P], BF16, tag="ps", name="ps")
                    nc.tensor.transpose(
                        pst[:], r1b[:, k * P:(k + 1) * P], identb[:]
                    )
                    t = work.tile([P, P], BF16, tag=f"r1t{k}", name=f"r1t{k}")
                    nc.vector.tensor_copy(t[:], pst[:])
                    r1t.append(t)

            # ---- GCN1 local features + AllGather --------------------------
            ps_h = ps.tile([P, F1], dt, tag="ps", name="ps")
            for k in range(KX):
                nc.tensor.matmul(
                    ps_h[:], r1t[k][:],
                    pr_sb[:, PR_G1W + k * F1:PR_G1W + (k + 1) * F1],
                    start=(k == 0), stop=(k == KX - 1),
                )
            h1g_sb = work.tile([P, F1], BF16, tag="h1g", name="h1g")
            nc.vector.tensor_copy(h1g_sb[:], ps_h[:])
            cin1_t = dpool.tile([P, F1], BF16, tag="cin1", name="cin1")
            cout1_t = dpool.tile([NNODE, F1], BF16, tag="cout1", name="cout1")
            nc.scalar.dma_start(cin1_t[:], h1g_sb[:])
            nc.gpsimd.collective_compute(
                "AllGather",
                OP.bypass,
                replica_groups=[list(range(NCORE))],
                ins=[cin1_t[:].opt()],
                outs=[cout1_t[:].opt()],
            )

            # ---- independent work during AllGather #1 ---------------------
            ps_rw = ps.tile([P, F1], dt, tag="psrw", name="psrw", bufs=1)
            nc.tensor.matmul(
                ps_rw[:], rT_sb[:], pc3_sb[:, C3_WM12:C3_WM12 + F1]
            )
            ps_rw2 = ps.tile([P, F1], dt, tag="psrw2", name="psrw2", bufs=1)
            nc.tensor.matmul(
                ps_rw2[:], rT_sb[:], pc3_sb[:, C3_W2S:C3_W2S + F1]
            )
            ps_rm = ps.tile([P, F1], dt, tag="psrm", name="psrm", bufs=1)
            for k in range(KX):
                nc.tensor.matmul(
                    ps_rm[:], r1t[k][:],
                    pr_sb[:, PR_W13 + k * F1:PR_W13 + (k + 1) * F1],
                    start=(k == 0), stop=(k == KX - 1),
                )
            # keep-warm: depends on h1g so it runs inside the AG window
            for _ in range(N_WARM1):
                nc.tensor.matmul(spam[:], identb[:], h1g_sb[:, 0:P])

            # ---- GCN1 gather-side: out = A_norm^T h_all + b, relu ---------
            # split the gathered [1024,256] into two DMAs on two queues
            hallA = work.tile([P, 4 * F1], BF16, tag="hallA", name="hallA")
            hallB = work.tile([P, 4 * F1], BF16, tag="hallB", name="hallB")
            nc.sync.dma_start(
                hallA[:].rearrange("p (s f) -> p s f", s=4),
                cout1_t[:][0:4 * P, :].rearrange("(s p) f -> p s f", s=4),
            )
            nc.scalar.dma_start(
                hallB[:].rearrange("p (s f) -> p s f", s=4),
                cout1_t[:][4 * P:NNODE, :].rearrange("(s p) f -> p s f", s=4),
            )
            ps_o1 = ps.tile([P, F1], dt, tag="ps", name="ps")
            for s in range(NCORE):
                hsrc = hallA if s < 4 else hallB
                nc.tensor.matmul(
                    ps_o1[:],
                    pr_sb[:, PR_ABLK + s * P:PR_ABLK + (s + 1) * P],
                    hsrc[:, (s % 4) * F1:(s % 4 + 1) * F1],
                    start=(s == 0), stop=False,
                )
            nc.tensor.matmul(
                ps_o1[:], onesb[:], crow_sb[:, CR_G1B:CR_G1B + F1],
                start=False, stop=True,
            )
            h1r_sb = work.tile([P, F1], dt, tag="h1r", name="h1r")
            nc.scalar.activation(h1r_sb[:], ps_o1[:], AF.Relu)

            # ---- h1f = (r@wm12)*h1r + 2e-4*(res1@wm13); res2 --------------
            h1m_sb = work.tile([P, F1], dt, tag="h1m", name="h1m")
            nc.vector.tensor_tensor(h1m_sb[:], ps_rw[:], h1r_sb[:], op=OP.mult)
            h1f_sb = work.tile([P, F1], dt, tag="h1f", name="h1f")
            nc.vector.scalar_tensor_tensor(
                h1f_sb[:], ps_rm[:], 2e-4, h1m_sb[:], op0=OP.mult, op1=OP.add
            )
            res2b_sb = work.tile([P, F1], BF16, tag="res2b", name="res2b")
            nc.vector.tensor_tensor(
                res2b_sb[:], ps_rw2[:], h1f_sb[:], op=OP.mult
            )

            # ---- GCN2 local features + AllGather --------------------------
            r2t = []
            for c2 in range(2):
                ps_tr = ps.tile([P, P], BF16, tag="ps", name="ps")
                nc.tensor.transpose(
                    ps_tr[:], res2b_sb[:, c2 * P:(c2 + 1) * P], identb[:]
                )
                t = work.tile([P, P], BF16, tag=f"r2t{c2}", name=f"r2t{c2}")
                nc.vector.tensor_copy(t[:], ps_tr[:])
                r2t.append(t)
            ps_h2 = ps.tile([P, F2], dt, tag="ps", name="ps")
            for c2 in range(2):
                nc.tensor.matmul(
                    ps_h2[:], r2t[c2][:],
                    pr_sb[:, PR_G2W + c2 * F2:PR_G2W + (c2 + 1) * F2],
                    start=(c2 == 0), stop=(c2 == 1),
                )
            h2g_sb = work.tile([P, F2], BF16, tag="h2g", name="h2g")
            nc.vector.tensor_copy(h2g_sb[:], ps_h2[:])
            cin2_t = dpool.tile([P, F2], BF16, tag="cin2", name="cin2")
            cout2_t = dpool.tile([NNODE, F2], BF16, tag="cout2", name="cout2")
            nc.scalar.dma_start(cin2_t[:], h2g_sb[:])
            nc.gpsimd.collective_compute(
                "AllGather",
                OP.bypass,
                replica_groups=[list(range(NCORE))],
                ins=[cin2_t[:].opt()],
                outs=[cout2_t[:].opt()],
            )
            # keep-warm during AllGather #2
            for _ in range(N_WARM2):
                nc.tensor.matmul(spam[:, 0:F2], identb[:], h2g_sb[:])

            # ---- GCN2 gather-side + fc (log_softmax on host) --------------
            hall2 = work.tile([P, NCORE * F2], BF16, tag="hall2", name="hall2")
            nc.sync.dma_start(
                hall2[:].rearrange("p (s f) -> p s f", s=NCORE),
                cout2_t[:].rearrange("(s p) f -> p s f", s=NCORE),
            )
            ps_o2 = ps.tile([P, F2], dt, tag="ps", name="ps")
            for s in range(NCORE):
                nc.tensor.matmul(
                    ps_o2[:],
                    pr_sb[:, PR_ABLK + s * P:PR_ABLK + (s + 1) * P],
                    hall2[:, s * F2:(s + 1) * F2],
                    start=(s == 0), stop=False,
                )
            nc.tensor.matmul(
                ps_o2[:], onesb[:], crow_sb[:, CR_G2B:CR_G2B + F2],
                start=False, stop=True,
            )
            h2r_sb = work.tile([P, F2], BF16, tag="h2r", name="h2r")
            nc.scalar.activation(h2r_sb[:], ps_o2[:], AF.Relu)

            ps_t2 = ps.tile([F2, P], BF16, tag="ps", name="ps")
            nc.tensor.transpose(ps_t2[:], h2r_sb[:], identb[:])
            h2rT_sb = work.tile([F2, P], BF16, tag="h2rT", name="h2rT")
            nc.vector.tensor_copy(h2rT_sb[:], ps_t2[:])
            ps_z = ps.tile([P, 8], dt, tag="ps", name="ps")
            nc.tensor.matmul(ps_z[:], h2rT_sb[:], fcw_sb[:], start=True, stop=False)
            nc.tensor.matmul(
                ps_z[:], onesb[:], crow_sb[:, CR_FCB:CR_FCB + 8],
                start=False, stop=True,
            )
            o_sb = work.tile([P, 8], dt, tag="osb", name="osb")
            nc.vector.tensor_copy(o_sb[:], ps_z[:])
            nc.sync.dma_start(out_d[:, :], o_sb[:])

    nc.compile()
    return nc


def _get_module(T: int) -> "bacc.Bacc":
    if T not in _module_cache:
        _module_cache[T] = _build(T)
    return _module_cache[T]


def _f32c(a) -> np.ndarray:
    return np.ascontiguousarray(np.asarray(a, dtype=np.float32))


def _bf16c(a) -> np.ndarray:
    return np.ascontiguousarray(
        np.asarray(a, dtype=np.float32).astype(NPBF16)
    )


def _fm(x):
    """[128, D] node-rows -> feature-major [128, D] block layout:
    out[p, k*128+n] = x[n, k*128+p]."""
    d = x.shape[1]
    kx = d // P
    return x.T.reshape(kx, P, P).transpose(1, 0, 2).reshape(P, kx * P)


def _wk(w):
    """[D, O] weight -> k-tiles side by side: out[p, k*O+o] = w[k*128+p, o]."""
    d, o = w.shape
    kx = d // P
    return w.reshape(kx, P, o).transpose(1, 0, 2).reshape(P, kx * o)


def _prepare(inputs):
    f = {k: np.asarray(v) for k, v in inputs.items()}
    x1, x11, x2 = f["x1"], f["x11"], f["x2"]
    edge = np.asarray(f["edge_index"]).astype(np.int64)
    src, dst = edge[0], edge[1]

    # host-side weight-only reductions
    W1s = np.asarray(f["wm1"], np.float32).sum(-1)        # [3, 768]
    C2s = np.asarray(f["class2"], np.float32).sum(-1)     # [2, 768]
    W2s = np.asarray(f["wm2"], np.float32).sum(-1)        # [3, 256]

    # degree normalization, folded into the adjacency blocks
    deg = np.bincount(dst, minlength=NNODE).astype(np.float32)
    dinv = (deg > 0) / np.sqrt(np.maximum(deg, 1.0))

    pma_shared = _wk(np.float32(f["mlp_W1"]))
    pmb = np.concatenate(
        [_wk(np.float32(f["mlp_W2"])), _wk(np.float32(f["mlp_W3"]))], axis=1
    ).astype(np.float16)
    pga_shared = _wk(np.float32(f["m1_W1"]))
    pgb = np.concatenate(
        [_wk(np.float32(f["m1_W2"])), _wk(np.float32(f["m1_W3"]))], axis=1
    ).astype(NPBF16)
    pr_shared = np.concatenate(
        [
            _wk(np.float32(f["gcn1_W"])),
            _wk(np.float32(f["wm13"])),
        ],
        axis=1,
    )
    g2w = _wk(np.float32(f["gcn2_W"]))                        # [128, 64]

    pc3 = np.zeros((3, C3_END), np.float32)
    pc3[:, C3_BP2:C3_BP2 + 768] = np.float32(f["bp2"])
    pc3[:, C3_WM12:C3_WM12 + 256] = np.float32(f["wm12"])
    pc3[:, C3_W1S:C3_W1S + 768] = W1s
    pc3[:, C3_W2S:C3_W2S + 256] = W2s

    pc2 = np.zeros((2, C2_END), np.float32)
    pc2[:, C2_BP1:C2_BP1 + 768] = np.float32(f["bp1"])
    pc2[:, C2_C2S:C2_C2S + 768] = C2s

    crow = np.zeros((1, CR_END), np.float32)
    crow[0, CR_G1B:CR_G1B + F1] = np.float32(f["gcn1_b"])
    crow[0, CR_G2B:CR_G2B + F2] = np.float32(f["gcn2_b"])
    crow[0, CR_FCB:CR_FCB + 8] = np.float32(f["fc_b"])
    crow[0, CR_GB3:CR_GB3 + 2] = np.float32(f["m1_b3"])
    crow[0, CR_MB1:CR_MB1 + H] = np.float32(f["mlp_b1"])
    crow[0, CR_MB2:CR_MB2 + H] = np.float32(f["mlp_b2"])
    crow[0, CR_GB1:CR_GB1 + H] = np.float32(f["m1_b1"])
    crow[0, CR_GB2:CR_GB2 + H] = np.float32(f["m1_b2"])
    crow[0, CR_MB3:CR_MB3 + 3] = np.float32(f["mlp_b3"])

    shared = {
        "pmb": pmb,
        "pgb": np.ascontiguousarray(pgb),
        "pc3": _bf16c(pc3),
        "pc2": _bf16c(pc2),
        "crow": _bf16c(crow),
        "fcw": _bf16c(f["fc_W"]),
    }

    # per-(dst-shard, src-shard) edge buckets
    csh = dst // P
    ssh = src // P
    srcl = src % P
    dstl = dst % P

    in_maps = []
    for c in range(NCORE):
        rows = slice(c * P, (c + 1) * P)
        ablk = np.zeros((NCORE, P, P), np.float32)
        m = csh == c
        np.add.at(ablk, (ssh[m], srcl[m], dstl[m]), 1.0)
        ablk *= dinv.reshape(NCORE, P)[:, :, None]            # dinv[src]
        ablk *= dinv[rows][None, None, :]                     # dinv[dst]
        ablk_cols = ablk.transpose(1, 0, 2).reshape(P, NCORE * P)

        mm = dict(shared)
        mm["pma"] = np.ascontiguousarray(
            np.concatenate(
                [_fm(np.float32(x1[rows])), pma_shared, np.float32(x2[rows])],
                axis=1,
            ).astype(np.float16)
        )
        mm["pga"] = np.concatenate(
            [_fm(np.float32(x11[rows])), pga_shared], axis=1
        ).astype(NPBF16)
        mm["pr"] = np.concatenate(
            [pr_shared, ablk_cols, g2w], axis=1
        ).astype(NPBF16)
        in_maps.append(mm)
    return 0, in_maps


def run(inputs, trace=False, **kw):
    """Full pipeline; returns (output [1024,8] f32, BassKernelResults)."""
    T, in_maps = _prepare(inputs)
    nc = _get_module(T)
    res = bass_utils.run_bass_kernel_spmd(
        nc, in_maps, core_ids=list(range(NCORE)), trace=trace, **kw
    )
    z = np.concatenate(
        [res.results[c]["out"] for c in range(NCORE)], axis=0
    ).astype(np.float64)
    out = (z - np.log(np.exp(z).sum(axis=1, keepdims=True))).astype(np.float32)
    return out, res


def kernel(**inputs) -> np.ndarray:
    out, _ = run(inputs)
    return out


# revision 28
# speedup vs baseline: 1.2709x; 1.2709x over previous
"""Trainium2 Bass kernel for nn_CustomModel_13657996001613 (moe_routing).

Distribution: data-parallel over nodes (1024 nodes -> 8 cores x 128).
GCN segment-sums use host-bucketed edges folded (with the degree
normalization) into 128x128 adjacency blocks, plus one AllGather of the
per-shard GCN features per GCN layer.

Host precomputes the weight-only reductions (sum_l wm1 / class2 / wm2)
and packs all device inputs into a handful of [128, W] buffers so the
kernel issues only a few large DMAs.  The logits MLP runs in fp16 (its
argmax decides routing; fp16 noise is well under the observed top-2
logit gaps); everything else off the argmax path is bf16.  The final
log_softmax normalization happens on host (rowwise, exact).
"""
import sys

if "/opt/trn_rl_repo" not in sys.path:
    sys.path.insert(0, "/opt/trn_rl_repo")

import ml_dtypes
import numpy as np

import concourse.bass as bass  # noqa: F401
import concourse.mybir as mybir
import concourse.tile as tile
from concourse import bacc, bass_utils
from concourse.masks import make_identity

F32 = mybir.dt.float32
F16 = mybir.dt.float16
BF16 = mybir.dt.bfloat16
NPBF16 = ml_dtypes.bfloat16
OP = mybir.AluOpType
AF = mybir.ActivationFunctionType
AX = mybir.AxisListType

NCORE = 8
P = 128
NNODE = 1024
D = 768
KX = D // P      # 6
H = 512
HT = H // P      # 4
F1 = 256
F2 = 32
TAU = 0.7
RLO = (1.0 - TAU) / 2.0       # 0.15
RDELT = TAU - RLO             # 0.55

# ---- packed-buffer column offsets (shared between host and device) ----
# pma (f16): x1fm | mW1 | x2nm     pmb (f16): mW2 | mW3
PMA_X1, PMA_W1 = 0, 768
PMA_X2 = PMA_W1 + KX * H
PMA_END = PMA_X2 + 768
PMB_W2, PMB_W3, PMB_END = 0, HT * H, HT * H + HT * 3
# pga (bf16): x11fm | gW1          pgb (bf16): gW2 | gW3
PGA_X11, PGA_W1, PGA_END = 0, 768, 768 + KX * H
PGB_W2, PGB_W3, PGB_END = 0, HT * H, HT * H + HT * 2
# pr (bf16): g1W | w13 | ablk | g2W
PR_G1W = 0
PR_W13 = PR_G1W + KX * F1
PR_ABLK = PR_W13 + KX * F1
PR_G2W = PR_ABLK + NCORE * P
PR_END = PR_G2W + 2 * F2
# pc (f32, [128, *]): per-tile bias columns mb1 | mb2 | gb1 | gb2
PC_MB1, PC_MB2, PC_GB1, PC_GB2, PC_END = 0, 4, 8, 12, 16
# pc3 (bf16, [3, *]): bp2 | wm12 | W1s | W2s
C3_BP2, C3_WM12, C3_W1S, C3_W2S, C3_END = 0, 768, 1024, 1792, 2048
# pc2 (bf16, [2, *]): bp1 | C2s
C2_BP1, C2_C2S, C2_END = 0, 768, 1536
# crow (bf16, [1, *]): g1b | g2b | fcb | gb3 | mb3
CR_G1B, CR_G2B, CR_FCB, CR_GB3, CR_MB3, CR_END = 0, 256, 288, 296, 298, 301

N_WARM = 40      # PE warm-up matmuls at t=0
N_WARM1 = 24     # keep-warm matmuls during AllGather #1
N_WARM2 = 12     # keep-warm matmuls during AllGather #2

_module_cache: dict[int, "bacc.Bacc"] = {}


def _build(T: int) -> "bacc.Bacc":
    nc = bacc.Bacc(
        "TRN2",
        target_bir_lowering=False,
        debug=False,
        enable_asserts=False,
        num_devices=NCORE,
    )
    dt = F32

    pma_d = nc.dram_tensor("pma", [P, PMA_END], F16, kind="ExternalInput")
    pmb_d = nc.dram_tensor("pmb", [P, PMB_END], F16, kind="ExternalInput")
    pga_d = nc.dram_tensor("pga", [P, PGA_END], BF16, kind="ExternalInput")
    pgb_d = nc.dram_tensor("pgb", [P, PGB_END], BF16, kind="ExternalInput")
    pr_d = nc.dram_tensor("pr", [P, PR_END], BF16, kind="ExternalInput")
    pc_d = nc.dram_tensor("pc", [P, PC_END], F32, kind="ExternalInput")
    pc3_d = nc.dram_tensor("pc3", [3, C3_END], BF16, kind="ExternalInput")
    pc2_d = nc.dram_tensor("pc2", [2, C2_END], BF16, kind="ExternalInput")
    crow_d = nc.dram_tensor("crow", [1, CR_END], BF16, kind="ExternalInput")
    fcw_d = nc.dram_tensor("fcw", [F2, 8], BF16, kind="ExternalInput")
    out_d = nc.dram_tensor("out", [P, 8], F32, kind="ExternalOutput")

    with tile.TileContext(nc) as tc:
        from contextlib import ExitStack

        ctx = ExitStack()
        with ctx:
            const = ctx.enter_context(tc.tile_pool(name="const", bufs=1))
            packs = ctx.enter_context(tc.tile_pool(name="packs", bufs=1))
            actp = ctx.enter_context(tc.tile_pool(name="actp", bufs=9))
            work = ctx.enter_context(tc.tile_pool(name="work", bufs=1))
            ps = ctx.enter_context(tc.tile_pool(name="ps", bufs=4, space="PSUM"))
            psw = ctx.enter_context(tc.tile_pool(name="psw", bufs=1, space="PSUM"))
            dpool = ctx.enter_context(
                tc.tile_pool(name="dram", bufs=1, space="DRAM")
            )

            # ---- constants ------------------------------------------------
            identb = const.tile([P, P], BF16, tag="identb", name="identb")
            make_identity(nc, identb[:])
            onesb = const.tile([1, P], BF16, tag="onesb", name="onesb")
            nc.vector.memset(onesb[:], 1.0)
            junk = const.tile([1, 1], dt, tag="junk", name="junk")
            nc.vector.memset(junk[:], 0.25)

            # ---- small-constant DMAs (scalar queue) -----------------------
            pc_sb = const.tile([P, PC_END], dt, tag="pc", name="pc")
            nc.scalar.dma_start(pc_sb[:], pc_d[:, :])
            pc3_sb = const.tile([3, C3_END], BF16, tag="pc3", name="pc3")
            nc.scalar.dma_start(pc3_sb[:], pc3_d[:, :])
            pc2_sb = const.tile([2, C2_END], BF16, tag="pc2", name="pc2")
            nc.scalar.dma_start(pc2_sb[:], pc2_d[:, :])
            crow_sb = const.tile([1, CR_END], BF16, tag="crow", name="crow")
            nc.scalar.dma_start(crow_sb[:], crow_d[:, :])
            fcw_sb = const.tile([F2, 8], BF16, tag="fcw", name="fcw")
            nc.scalar.dma_start(fcw_sb[:], fcw_d[:, :])

            # pre-load the Relu activation table while the queue is idle
            jout = work.tile([1, 1], dt, tag="jout", name="jout")
            nc.scalar.activation(jout[:], junk[:], AF.Relu)

            # ---- big input DMAs, dependency order (sync queue) ------------
            pga_sb = packs.tile([P, PGA_END], BF16, tag="pga", name="pga")
            nc.sync.dma_start(pga_sb[:], pga_d[:, :])
            pma_sb = packs.tile([P, PMA_END], F16, tag="pma", name="pma")
            nc.sync.dma_start(pma_sb[:], pma_d[:, :])
            pgb_sb = packs.tile([P, PGB_END], BF16, tag="pgb", name="pgb")
            nc.sync.dma_start(pgb_sb[:], pgb_d[:, :])
            pmb_sb = packs.tile([P, PMB_END], F16, tag="pmb", name="pmb")
            nc.sync.dma_start(pmb_sb[:], pmb_d[:, :])
            pr_sb = packs.tile([P, PR_END], BF16, tag="pr", name="pr")
            nc.sync.dma_start(pr_sb[:], pr_d[:, :])

            # ---- PE warm-up spam (runs while DMAs stream) -----------------
            spam = psw.tile([P, P], dt, tag="spam", name="spam")
            for _ in range(N_WARM):
                nc.tensor.matmul(spam[:], identb[:], identb[:])

            # ---- 2-hidden-layer MLP, feature-major ------------------------
            # Weights are the stationary operand, activations stream as rhs;
            # hidden tiles come out feature-major [128h, 128n] directly, so
            # no transposes are needed.  Bias+Relu fuse into the per-tile
            # psum->sbuf ACT (bias is per-partition in this orientation).
            def mlp_layer(xs, kx, woff, boff, pw, wdt):
                hs = []
                for j in range(HT):
                    psj = ps.tile([P, P], dt, tag="ps", name="ps")
                    for k in range(kx):
                        nc.tensor.matmul(
                            psj[:],
                            pw[:, woff + k * H + j * P:woff + k * H + (j + 1) * P],
                            xs[k],
                            start=(k == 0), stop=(k == kx - 1),
                        )
                    t = actp.tile([P, P], wdt, tag="ht", name="ht")
                    nc.scalar.activation(
                        t[:], psj[:], AF.Relu,
                        bias=pc_sb[:, boff + j:boff + j + 1],
                    )
                    hs.append(t)
                return hs

            def mlp_2layers(xoff, w1off, w2off, pa, pb, b1off, b2off, wdt):
                xs = [
                    pa[:, xoff + k * P:xoff + (k + 1) * P] for k in range(KX)
                ]
                h1 = mlp_layer(xs, KX, w1off, b1off, pa, wdt)
                return mlp_layer(
                    [t[:] for t in h1], HT, w2off, b2off, pb, wdt
                )

            # ---- branch g: x11 -> gT [2,128] bf16 -------------------------
            g_h2 = mlp_2layers(
                PGA_X11, PGA_W1, PGB_W2, pga_sb, pgb_sb,
                PC_GB1, PC_GB2, BF16,
            )
            ps_g = ps.tile([2, P], dt, tag="ps", name="ps")
            for k in range(HT):
                nc.tensor.matmul(
                    ps_g[:],
                    pgb_sb[:, PGB_W3 + k * 2:PGB_W3 + (k + 1) * 2],
                    g_h2[k][:],
                    start=(k == 0), stop=False,
                )
            nc.tensor.matmul(
                ps_g[:], crow_sb[:, CR_GB3:CR_GB3 + 2], onesb[:],
                start=False, stop=True,
            )
            gT_sb = work.tile([2, P], BF16, tag="gT", name="gT")
            nc.vector.tensor_copy(gT_sb[:], ps_g[:])

            # ---- branch logits (fp16): x1 -> r -> rT [3,128] bf16 ---------
            m_h2 = mlp_2layers(
                PMA_X1, PMA_W1, PMB_W2, pma_sb, pmb_sb,
                PC_MB1, PC_MB2, F16,
            )
            ps_l = ps.tile([P, 3], dt, tag="ps", name="ps")
            for k in range(HT):
                nc.tensor.matmul(
                    ps_l[:], m_h2[k][:],
                    pmb_sb[:, PMB_W3 + k * 3:PMB_W3 + (k + 1) * 3],
                    start=(k == 0), stop=False,
                )
            nc.tensor.matmul(
                ps_l[:], onesb[:], crow_sb[:, CR_MB3:CR_MB3 + 3],
                start=False, stop=True,
            )
            mx_sb = work.tile([P, 1], dt, tag="mx", name="mx")
            nc.vector.tensor_reduce(mx_sb[:], ps_l[:], axis=AX.X, op=OP.max)
            r_sb = work.tile([P, 3], BF16, tag="r", name="r")
            nc.vector.tensor_scalar(
                r_sb[:], ps_l[:], mx_sb[:, 0:1], None, OP.is_ge
            )
            nc.vector.tensor_scalar(
                r_sb[:], r_sb[:], RDELT, RLO, OP.mult, OP.add
            )
            ps_rt = ps.tile([3, P], BF16, tag="ps", name="ps")
            nc.tensor.transpose(ps_rt[:], r_sb[:], identb[:])
            rT_sb = work.tile([3, P], BF16, tag="rT", name="rT")
            nc.vector.tensor_copy(rT_sb[:], ps_rt[:])

            # ---- res1 chain, node-major, half-pipelined -------------------
            # res1 = ((C2s^T g) * x2 + bp1^T g) * (W1s^T r) + bp2^T r
            r1 = work.tile([P, D], dt, tag="r1", name="r1")
            r1b = work.tile([P, D], BF16, tag="r1b", name="r1b")
            r1t = []
            HW = 384
            for h_ in range(2):
                sl = slice(h_ * HW, (h_ + 1) * HW)
                psT1 = ps.tile([P, HW], dt, tag="ps", name="ps")
                nc.tensor.matmul(
                    psT1[:], gT_sb[:], pc2_sb[:, C2_C2S + h_ * HW:C2_C2S + (h_ + 1) * HW]
                )
                psT2 = ps.tile([P, HW], dt, tag="ps", name="ps")
                nc.tensor.matmul(
                    psT2[:], gT_sb[:], pc2_sb[:, C2_BP1 + h_ * HW:C2_BP1 + (h_ + 1) * HW]
                )
                psT3 = ps.tile([P, HW], dt, tag="ps", name="ps")
                nc.tensor.matmul(
                    psT3[:], rT_sb[:], pc3_sb[:, C3_W1S + h_ * HW:C3_W1S + (h_ + 1) * HW]
                )
                psT4 = ps.tile([P, HW], dt, tag="ps", name="ps")
                nc.tensor.matmul(
                    psT4[:], rT_sb[:], pc3_sb[:, C3_BP2 + h_ * HW:C3_BP2 + (h_ + 1) * HW]
                )
                nc.vector.tensor_tensor(
                    r1[:, sl], psT1[:], pma_sb[:, PMA_X2 + h_ * HW:PMA_X2 + (h_ + 1) * HW],
                    op=OP.mult,
                )
                nc.vector.tensor_tensor(r1[:, sl], r1[:, sl], psT2[:], op=OP.add)
                nc.vector.tensor_tensor(r1[:, sl], r1[:, sl], psT3[:], op=OP.mult)
                nc.vector.tensor_tensor(r1b[:, sl], r1[:, sl], psT4[:], op=OP.add)
                for k in range(h_ * 3, h_ * 3 + 3):
                    pst = ps.tile([P, P], BF16, tag="ps", name="ps")
                    nc.tensor.transpose(
                        pst[:], r1b[:, k * P:(k + 1) * P], identb[:]
                    )
                    t = work.tile([P, P], BF16, tag=f"r1t{k}", name=f"r1t{k}")
                    nc.vector.tensor_copy(t[:], pst[:])
                    r1t.append(t)

            # ---- GCN1 local features + AllGather --------------------------
            ps_h = ps.tile([P, F1], dt, tag="ps", name="ps")
            for k in range(KX):
                nc.tensor.matmul(
                    ps_h[:], r1t[k][:],
                    pr_sb[:, PR_G1W + k * F1:PR_G1W + (k + 1) * F1],
                    start=(k == 0), stop=(k == KX - 1),
                )
            h1g_sb = work.tile([P, F1], BF16, tag="h1g", name="h1g")
            nc.vector.tensor_copy(h1g_sb[:], ps_h[:])
            cin1_t = dpool.tile([P, F1], BF16, tag="cin1", name="cin1")
            cout1_t = dpool.tile([NNODE, F1], BF16, tag="cout1", name="cout1")
            nc.scalar.dma_start(cin1_t[:], h1g_sb[:])
            nc.gpsimd.collective_compute(
                "AllGather",
                OP.bypass,
                replica_groups=[list(range(NCORE))],
                ins=[cin1_t[:].opt()],
                outs=[cout1_t[:].opt()],
            )

            # ---- independent work during AllGather #1 ---------------------
            # res2 = (r@W2s) * [(r@wm12)*relu(o1) + 2e-4*(res1@wm13)]
            #      = Ap*relu(o1) + Bp  with  Ap=(r@wm12)*(r@W2s),
            #        Bp=2e-4*(res1@wm13)*(r@W2s) — both free of o1, so they
            #        compute inside the AG window
            ps_rw = ps.tile([P, F1], dt, tag="psrw", name="psrw", bufs=1)
            nc.tensor.matmul(
                ps_rw[:], rT_sb[:], pc3_sb[:, C3_WM12:C3_WM12 + F1]
            )
            ps_rw2 = ps.tile([P, F1], dt, tag="psrw2", name="psrw2", bufs=1)
            nc.tensor.matmul(
                ps_rw2[:], rT_sb[:], pc3_sb[:, C3_W2S:C3_W2S + F1]
            )
            ps_rm = ps.tile([P, F1], dt, tag="psrm", name="psrm", bufs=1)
            for k in range(KX):
                nc.tensor.matmul(
                    ps_rm[:], r1t[k][:],
                    pr_sb[:, PR_W13 + k * F1:PR_W13 + (k + 1) * F1],
                    start=(k == 0), stop=(k == KX - 1),
                )
            rw2_sb = work.tile([P, F1], dt, tag="rw2s", name="rw2s")
            nc.vector.tensor_copy(rw2_sb[:], ps_rw2[:])
            ap_sb = work.tile([P, F1], dt, tag="apm", name="apm")
            nc.vector.tensor_tensor(ap_sb[:], ps_rw[:], rw2_sb[:], op=OP.mult)
            bp_sb = work.tile([P, F1], dt, tag="bpm", name="bpm")
            nc.vector.scalar_tensor_tensor(
                bp_sb[:], ps_rm[:], 2e-4, rw2_sb[:], op0=OP.mult, op1=OP.mult
            )
            # keep-warm: depends on h1g so it runs inside the AG window
            for _ in range(N_WARM1):
                nc.tensor.matmul(spam[:], identb[:], h1g_sb[:, 0:P])

            # ---- GCN1 gather-side: out = A_norm^T h_all + b, relu ---------
            # split the gathered [1024,256] into four DMAs on three queues
            halls = []
            for q, eng in enumerate([nc.sync, nc.scalar, nc.gpsimd, nc.sync]):
                hq = work.tile([P, 2 * F1], BF16, tag=f"hall{q}", name=f"hall{q}")
                eng.dma_start(
                    hq[:].rearrange("p (s f) -> p s f", s=2),
                    cout1_t[:][q * 2 * P:(q + 1) * 2 * P, :].rearrange(
                        "(s p) f -> p s f", s=2
                    ),
                )
                halls.append(hq)
            ps_o1 = ps.tile([P, F1], dt, tag="ps", name="ps")
            for s in range(NCORE):
                nc.tensor.matmul(
                    ps_o1[:],
                    pr_sb[:, PR_ABLK + s * P:PR_ABLK + (s + 1) * P],
                    halls[s // 2][:, (s % 2) * F1:(s % 2 + 1) * F1],
                    start=(s == 0), stop=False,
                )
            nc.tensor.matmul(
                ps_o1[:], onesb[:], crow_sb[:, CR_G1B:CR_G1B + F1],
                start=False, stop=True,
            )
            h1r_sb = work.tile([P, F1], dt, tag="h1r", name="h1r")
            nc.scalar.activation(h1r_sb[:], ps_o1[:], AF.Relu)

            # ---- res2 = Ap*relu(o1) + Bp ----------------------------------
            h1m_sb = work.tile([P, F1], dt, tag="h1m", name="h1m")
            nc.vector.tensor_tensor(h1m_sb[:], ap_sb[:], h1r_sb[:], op=OP.mult)
            res2b_sb = work.tile([P, F1], BF16, tag="res2b", name="res2b")
            nc.vector.tensor_tensor(
                res2b_sb[:], h1m_sb[:], bp_sb[:], op=OP.add
            )

            # ---- GCN2 local features + AllGather --------------------------
            r2t = []
            for c2 in range(2):
                ps_tr = ps.tile([P, P], BF16, tag="ps", name="ps")
                nc.tensor.transpose(
                    ps_tr[:], res2b_sb[:, c2 * P:(c2 + 1) * P], identb[:]
                )
                t = work.tile([P, P], BF16, tag=f"r2t{c2}", name=f"r2t{c2}")
                nc.vector.tensor_copy(t[:], ps_tr[:])
                r2t.append(t)
            ps_h2 = ps.tile([P, F2], dt, tag="ps", name="ps")
            for c2 in range(2):
                nc.tensor.matmul(
                    ps_h2[:], r2t[c2][:],
                    pr_sb[:, PR_G2W + c2 * F2:PR_G2W + (c2 + 1) * F2],
                    start=(c2 == 0), stop=(c2 == 1),
                )
            h2g_sb = work.tile([P, F2], BF16, tag="h2g", name="h2g")
            nc.vector.tensor_copy(h2g_sb[:], ps_h2[:])
            cin2_t = dpool.tile([P, F2], BF16, tag="cin2", name="cin2")
            cout2_t = dpool.tile([NNODE, F2], BF16, tag="cout2", name="cout2")
            nc.scalar.dma_start(cin2_t[:], h2g_sb[:])
            nc.gpsimd.collective_compute(
                "AllGather",
                OP.bypass,
                replica_groups=[list(range(NCORE))],
                ins=[cin2_t[:].opt()],
                outs=[cout2_t[:].opt()],
            )
            # keep-warm during AllGather #2
            for _ in range(N_WARM2):
                nc.tensor.matmul(spam[:, 0:F2], identb[:], h2g_sb[:])

            # ---- GCN2 gather-side + fc (log_softmax on host) --------------
            hall2a = work.tile([P, 4 * F2], BF16, tag="hall2a", name="hall2a")
            hall2b = work.tile([P, 4 * F2], BF16, tag="hall2b", name="hall2b")
            nc.sync.dma_start(
                hall2a[:].rearrange("p (s f) -> p s f", s=4),
                cout2_t[:][0:4 * P, :].rearrange("(s p) f -> p s f", s=4),
            )
            nc.scalar.dma_start(
                hall2b[:].rearrange("p (s f) -> p s f", s=4),
                cout2_t[:][4 * P:NNODE, :].rearrange("(s p) f -> p s f", s=4),
            )
            ps_o2 = ps.tile([P, F2], dt, tag="ps", name="ps")
            for s in range(NCORE):
                h2src = hall2a if s < 4 else hall2b
                nc.tensor.matmul(
                    ps_o2[:],
                    pr_sb[:, PR_ABLK + s * P:PR_ABLK + (s + 1) * P],
                    h2src[:, (s % 4) * F2:(s % 4 + 1) * F2],
                    start=(s == 0), stop=False,
                )
            nc.tensor.matmul(
                ps_o2[:], onesb[:], crow_sb[:, CR_G2B:CR_G2B + F2],
                start=False, stop=True,
            )
            h2r_sb = work.tile([P, F2], BF16, tag="h2r", name="h2r")
            nc.scalar.activation(h2r_sb[:], ps_o2[:], AF.Relu)

            ps_t2 = ps.tile([F2, P], BF16, tag="ps", name="ps")
            nc.tensor.transpose(ps_t2[:], h2r_sb[:], identb[:])
            h2rT_sb = work.tile([F2, P], BF16, tag="h2rT", name="h2rT")
            nc.vector.tensor_copy(h2rT_sb[:], ps_t2[:])
            ps_z = ps.tile([P, 8], dt, tag="ps", name="ps")
            nc.tensor.matmul(ps_z[:], h2rT_sb[:], fcw_sb[:], start=True, stop=False)
            nc.tensor.matmul(
                ps_z[:], onesb[:], crow_sb[:, CR_FCB:CR_FCB + 8],
                start=False, stop=True,
            )
            o_sb = work.tile([P, 8], dt, tag="osb", name="osb")
            nc.vector.tensor_copy(o_sb[:], ps_z[:])
            nc.sync.dma_start(out_d[:, :], o_sb[:])

    nc.compile()
    return nc


def _get_module(T: int) -> "bacc.Bacc":
    if T not in _module_cache:
        _module_cache[T] = _build(T)
    return _module_cache[T]


def _f32c(a) -> np.ndarray:
    return np.ascontiguousarray(np.asarray(a, dtype=np.float32))


def _bf16c(a) -> np.ndarray:
    return np.ascontiguousarray(
        np.asarray(a, dtype=np.float32).astype(NPBF16)
    )


def _fm(x):
    """[128, D] node-rows -> feature-major [128, D] block layout:
    out[p, k*128+n] = x[n, k*128+p]."""
    d = x.shape[1]
    kx = d // P
    return x.T.reshape(kx, P, P).transpose(1, 0, 2).reshape(P, kx * P)


def _wk(w):
    """[D, O] weight -> k-tiles side by side: out[p, k*O+o] = w[k*128+p, o]."""
    d, o = w.shape
    kx = d // P
    return w.reshape(kx, P, o).transpose(1, 0, 2).reshape(P, kx * o)


def _prepare(inputs):
    f = {k: np.asarray(v) for k, v in inputs.items()}
    x1, x11, x2 = f["x1"], f["x11"], f["x2"]
    edge = np.asarray(f["edge_index"]).astype(np.int64)
    src, dst = edge[0], edge[1]

    # host-side weight-only reductions
    W1s = np.asarray(f["wm1"], np.float32).sum(-1)        # [3, 768]
    C2s = np.asarray(f["class2"], np.float32).sum(-1)     # [2, 768]
    W2s = np.asarray(f["wm2"], np.float32).sum(-1)        # [3, 256]

    # degree normalization, folded into the adjacency blocks
    deg = np.bincount(dst, minlength=NNODE).astype(np.float32)
    dinv = (deg > 0) / np.sqrt(np.maximum(deg, 1.0))

    pma_shared = _wk(np.float32(f["mlp_W1"]))
    pmb = np.concatenate(
        [_wk(np.float32(f["mlp_W2"])), _wk(np.float32(f["mlp_W3"]))], axis=1
    ).astype(np.float16)
    pga_shared = _wk(np.float32(f["m1_W1"]))
    pgb = np.concatenate(
        [_wk(np.float32(f["m1_W2"])), _wk(np.float32(f["m1_W3"]))], axis=1
    ).astype(NPBF16)
    pr_shared = np.concatenate(
        [
            _wk(np.float32(f["gcn1_W"])),
            _wk(np.float32(f["wm13"])),
        ],
        axis=1,
    )
    g2w = _wk(np.float32(f["gcn2_W"]))                        # [128, 64]

    pc = np.zeros((P, PC_END), np.float32)
    pc[:, PC_MB1:PC_MB1 + 4] = np.float32(f["mlp_b1"]).reshape(HT, P).T
    pc[:, PC_MB2:PC_MB2 + 4] = np.float32(f["mlp_b2"]).reshape(HT, P).T
    pc[:, PC_GB1:PC_GB1 + 4] = np.float32(f["m1_b1"]).reshape(HT, P).T
    pc[:, PC_GB2:PC_GB2 + 4] = np.float32(f["m1_b2"]).reshape(HT, P).T

    pc3 = np.zeros((3, C3_END), np.float32)
    pc3[:, C3_BP2:C3_BP2 + 768] = np.float32(f["bp2"])
    pc3[:, C3_WM12:C3_WM12 + 256] = np.float32(f["wm12"])
    pc3[:, C3_W1S:C3_W1S + 768] = W1s
    pc3[:, C3_W2S:C3_W2S + 256] = W2s

    pc2 = np.zeros((2, C2_END), np.float32)
    pc2[:, C2_BP1:C2_BP1 + 768] = np.float32(f["bp1"])
    pc2[:, C2_C2S:C2_C2S + 768] = C2s

    crow = np.zeros((1, CR_END), np.float32)
    crow[0, CR_G1B:CR_G1B + F1] = np.float32(f["gcn1_b"])
    crow[0, CR_G2B:CR_G2B + F2] = np.float32(f["gcn2_b"])
    crow[0, CR_FCB:CR_FCB + 8] = np.float32(f["fc_b"])
    crow[0, CR_GB3:CR_GB3 + 2] = np.float32(f["m1_b3"])
    crow[0, CR_MB3:CR_MB3 + 3] = np.float32(f["mlp_b3"])

    shared = {
        "pmb": pmb,
        "pgb": np.ascontiguousarray(pgb),
        "pc": _f32c(pc),
        "pc3": _bf16c(pc3),
        "pc2": _bf16c(pc2),
        "crow": _bf16c(crow),
        "fcw": _bf16c(f["fc_W"]),
    }

    # per-(dst-shard, src-shard) edge buckets
    csh = dst // P
    ssh = src // P
    srcl = src % P
    dstl = dst % P

    in_maps = []
    for c in range(NCORE):
        rows = slice(c * P, (c + 1) * P)
        ablk = np.zeros((NCORE, P, P), np.float32)
        m = csh == c
        np.add.at(ablk, (ssh[m], srcl[m], dstl[m]), 1.0)
        ablk *= dinv.reshape(NCORE, P)[:, :, None]            # dinv[src]
        ablk *= dinv[rows][None, None, :]                     # dinv[dst]
        ablk_cols = ablk.transpose(1, 0, 2).reshape(P, NCORE * P)

        mm = dict(shared)
        mm["pma"] = np.ascontiguousarray(
            np.concatenate(
                [_fm(np.float32(x1[rows])), pma_shared, np.float32(x2[rows])],
                axis=1,
            ).astype(np.float16)
        )
        mm["pga"] = np.concatenate(
            [_fm(np.float32(x11[rows])), pga_shared], axis=1
        ).astype(NPBF16)
        mm["pr"] = np.concatenate(
            [pr_shared, ablk_cols, g2w], axis=1
        ).astype(NPBF16)
        in_maps.append(mm)
    return 0, in_maps


def run(inputs, trace=False, **kw):
    """Full pipeline; returns (output [1024,8] f32, BassKernelResults)."""
    T, in_maps = _prepare(inputs)
    nc = _get_module(T)
    res = bass_utils.run_bass_kernel_spmd(
        nc, in_maps, core_ids=list(range(NCORE)), trace=trace, **kw
    )
    z = np.concatenate(
        [res.results[c]["out"] for c in range(NCORE)], axis=0
    ).astype(np.float64)
    out = (z - np.log(np.exp(z).sum(axis=1, keepdims=True))).astype(np.float32)
    return out, res


def kernel(**inputs) -> np.ndarray:
    out, _ = run(inputs)
    return out
